# revision 1
# baseline (speedup 1.0000x reference)
"""ChannelAttention (B,D,H,W,C = 4,8,32,32,512; 8 heads, head_dim 64) on 8
Trainium2 NeuronCores, Bass/Tile SPMD. Fully data-parallel: zero cross-core
communication.

Sharding: the 32768 tokens (B * D*H*W) are split 8 ways -> 4096 output tokens
per core; cores (2j, 2j+1) handle the two halves of batch j. Channel
attention needs the per-head 64x64 k^T v Gram matrix over ALL of a batch's
tokens, so each core redundantly computes k|v for its whole batch (8192
tokens; its own half ordered first in its xT input). This duplicated k|v work
(~55us of PE) is cheaper and far more robust than any cross-core reduction
(a ncfw AllReduce costs ~70us fixed + a 67us start barrier).

Schedule per core:
  pass A   : stream xT chunks (16 = both halves), k|v = x @ Wkv^T (k scale
             folded in on host), accumulate per-head-pair k^T v into PSUM
             (head-pair x head-quad blocking so fp32r matmuls hit the N>=256
             full-rate mode). HAM warm-up keeper matmuls run during the
             initial DMA fill.
  softmax  : pack 8 64x64 blocks, rowwise softmax over e on [128, 4, 64]
             (DVE reduce/reciprocal, ACT exp) -- overlapped with
  pass B   : qT = Wq @ x^T for the core's own 4096 tokens (re-streams x).
  phase 2  : PE-transpose probs into block-diagonal pair lhsT, out = attnT @
             qT, proj y = out @ Wproj^T (+bias via DVE broadcast add),
             software-pipelined one chunk ahead.

Numerics: all matmuls in float32r (fp32 storage, reduced-precision PE
multiply, ~13-bit effective mantissa) with fp32 PSUM accumulation; softmax in
fp32. End-to-end L2 relative error vs the fp32 reference: ~1.0e-3.
"""

import os
import numpy as np
from contextlib import ExitStack

import concourse.bass as bass
import concourse.mybir as mybir
import concourse.tile as tile
from concourse import bacc
from concourse.bass_utils import run_bass_kernel_spmd
from concourse.masks import make_identity

B, D, H, W, C = 4, 8, 32, 32, 512
NUM_HEADS = 8
HEAD_DIM = C // NUM_HEADS
SCALE = HEAD_DIM ** -0.5
N_TOK = B * D * H * W
N_CORES = 8
N_LOC = N_TOK // N_CORES
CHUNK = 512
N_CHUNKS = N_LOC // CHUNK
TT = 128
T_PER_CHUNK = CHUNK // TT
N_CI = C // 128
N_PAIRS = NUM_HEADS // 2

f32 = mybir.dt.float32
f32r = mybir.dt.float32r

N_KEEP_START = 24
N_KEEP_MID = 16

_NC_CACHE = None


def build_nc():
    nc = bacc.Bacc(num_devices=N_CORES)

    xT = nc.declare_dram_parameter("xT", [C, 2 * N_LOC], f32r, isOutput=False)
    wq = nc.declare_dram_parameter("wq", [C, C], f32r, isOutput=False)
    wkv = nc.declare_dram_parameter("wkv", [C, 2 * C], f32r, isOutput=False)
    wp = nc.declare_dram_parameter("wp", [C, C], f32r, isOutput=False)
    bp = nc.declare_dram_parameter("bp", [1, C], f32r, isOutput=False)
    y = nc.declare_dram_parameter("y", [N_LOC, C], f32, isOutput=True)

    xT_v = xT.rearrange("(a p) n -> p a n", p=128)
    wq_v = wq.rearrange("(a p) f -> p a f", p=128)
    wkv_v = wkv.rearrange("(a p) f -> p a f", p=128)
    wp_v = wp.rearrange("(a p) f -> p a f", p=128)

    with tile.TileContext(nc) as tc, ExitStack() as ctx:
        const = ctx.enter_context(tc.tile_pool(name="const", bufs=1))
        persist = ctx.enter_context(tc.tile_pool(name="persist", bufs=1))
        sb = ctx.enter_context(tc.tile_pool(name="sb", bufs=2))
        kvp = ctx.enter_context(tc.tile_pool(name="kvp", bufs=4))

        wkv_sb = const.tile([128, N_CI, 2 * C], f32r)
        nc.sync.dma_start(wkv_sb[:], wkv_v[:])
        ones_f32 = const.tile([1, 128], f32)
        nc.vector.memset(ones_f32[:], 1.0)
        ones_sb = const.tile([1, 128], f32r)
        nc.vector.tensor_copy(ones_sb[:], ones_f32[:])
        zrow_f32 = const.tile([1, 512], f32)
        nc.vector.memset(zrow_f32[:], 0.0)
        zrow_sb = const.tile([1, 512], f32r)
        nc.vector.tensor_copy(zrow_sb[:], zrow_f32[:])
        ident = const.tile([128, 128], f32)
        make_identity(nc, ident[:])

        qT_all = persist.tile([128, N_PAIRS, N_CHUNKS, CHUNK], f32r)
        red_sb = persist.tile([128, N_PAIRS, 64], f32)

        # ---------------- pass A: k|v + attn partial accumulation ----------
        with (
            tc.tile_pool(name="ps_kv", bufs=2, space="PSUM") as ps_kv,
            tc.tile_pool(name="ps_at", bufs=1, space="PSUM") as ps_at,
            tc.tile_pool(name="ps_q", bufs=2, space="PSUM") as ps_q,
        ):
            attn_ps = ps_at.tile([128, N_PAIRS, 256], f32)
            # bank-wide has_written seed + HAM warm-up
            for i in range(max(2, N_KEEP_START)):
                bank = i % 2
                nc.tensor.matmul(
                    attn_ps[:, 2 * bank:2 * bank + 2, :].rearrange("p a e -> p (a e)"),
                    ones_sb[:], zrow_sb[:],
                    start=(i < 2), stop=False, skip_group_check=True,
                )

            for c in range(2 * N_CHUNKS):
                xt = sb.tile([128, N_CI, CHUNK], f32r, tag="xt")
                nc.sync.dma_start(xt[:], xT_v[:, :, c * CHUNK:(c + 1) * CHUNK])

                kv_tiles = []
                for s in range(T_PER_CHUNK):
                    kv_ps = ps_kv.tile([128, 2 * C], f32, tag="kv")
                    for h in range(2):
                        for k in range(N_CI):
                            nc.tensor.matmul(
                                kv_ps[:, h * C:(h + 1) * C],
                                xt[:, k, s * TT:(s + 1) * TT],
                                wkv_sb[:, k, h * C:(h + 1) * C],
                                start=(k == 0), stop=(k == N_CI - 1),
                            )
                    kv_sb = kvp.tile([128, 2 * C], f32r, tag="kvsb")
                    nc.vector.tensor_copy(kv_sb[:], kv_ps[:])
                    kv_tiles.append(kv_sb)

                for s in range(T_PER_CHUNK):
                    kv_sb = kv_tiles[s]
                    for p in range(N_PAIRS):
                        q4 = p // 2
                        nc.tensor.matmul(
                            attn_ps[:, p, :],
                            kv_sb[:, p * 128:(p + 1) * 128],
                            kv_sb[:, C + q4 * 256:C + (q4 + 1) * 256],
                            start=False,
                            stop=(c == 2 * N_CHUNKS - 1 and s == T_PER_CHUNK - 1),
                            skip_group_check=True,
                        )

            # pack 8 useful 64x64 blocks -> red_sb[d + 64*(h%2), h//2, :]
            for h in range(NUM_HEADS):
                p = h // 2
                row0 = (h % 2) * 64
                col0 = (p % 2) * 128 + row0
                nc.vector.tensor_copy(
                    red_sb[row0:row0 + 64, h // 2, :],
                    attn_ps[row0:row0 + 64, p, col0:col0 + 64],
                )

            # weights for pass B / phase 2 (loaded once pass A's DMAs drain)
            wq_sb = const.tile([128, N_CI, C], f32r)
            nc.sync.dma_start(wq_sb[:], wq_v[:])
            wp_sb = const.tile([128, N_CI, C], f32r)
            nc.sync.dma_start(wp_sb[:], wp_v[:])
            bp_f32 = const.tile([128, C], f32)
            bp_bcast = bass.AP(
                tensor=bp[:].bitcast(f32).tensor,
                offset=0,
                ap=[[0, 128], [1, C]],
            )
            nc.sync.dma_start(bp_f32[:], bp_bcast)

            # ---- softmax over e on [128, pair, 64] (overlaps pass B) ----
            nmax = sb.tile([128, N_PAIRS, 1], f32, tag="nmax")
            nc.vector.reduce_max(nmax[:], red_sb[:], axis=mybir.AxisListType.X, negate=True)
            shifted = sb.tile([128, N_PAIRS, 64], f32, tag="shifted")
            nc.vector.tensor_add(shifted[:], red_sb[:], nmax.broadcast_to([128, N_PAIRS, 64]))
            expd = sb.tile([128, N_PAIRS, 64], f32, tag="expd")
            nc.scalar.activation(expd[:], shifted[:], mybir.ActivationFunctionType.Exp)
            ssum = sb.tile([128, N_PAIRS, 1], f32, tag="ssum")
            nc.vector.reduce_sum(ssum[:], expd[:], axis=mybir.AxisListType.X)
            rsum = sb.tile([128, N_PAIRS, 1], f32, tag="rsum")
            nc.vector.reciprocal(rsum[:], ssum[:])
            probs = sb.tile([128, N_PAIRS, 64], f32, tag="probs")
            nc.vector.tensor_mul(probs[:], expd[:], rsum.broadcast_to([128, N_PAIRS, 64]))
            probs2 = sb.tile([64, NUM_HEADS, 64], f32, tag="probs2")
            nc.vector.tensor_copy(probs2[:, 0::2, :], probs[0:64, :, :])
            nc.vector.tensor_copy(probs2[:, 1::2, :], probs[64:128, :, :])
            zro = sb.tile([128, N_PAIRS, 128], f32, tag="zro")
            nc.vector.memset(zro[:], 0.0)
            atnT = persist.tile([128, N_PAIRS, 128], f32r)
            nc.vector.tensor_copy(atnT[:], zro[:])

            # ------------- pass B: qT (overlaps the exchange) --------------
            for c in range(N_CHUNKS):
                xt = sb.tile([128, N_CI, CHUNK], f32r, tag="xtb")
                nc.sync.dma_start(xt[:], xT_v[:, :, c * CHUNK:(c + 1) * CHUNK])
                for p in range(N_PAIRS):
                    q_ps = ps_q.tile([128, CHUNK], f32, tag="q")
                    for k in range(N_CI):
                        nc.tensor.matmul(
                            q_ps[:],
                            wq_sb[:, k, p * 128:(p + 1) * 128],
                            xt[:, k, :],
                            start=(k == 0), stop=(k == N_CI - 1),
                        )
                    nc.scalar.copy(qT_all[:, p, c, :], q_ps[:])

        with (
            tc.tile_pool(name="ps_tr", bufs=1, space="PSUM") as ps_tr,
            tc.tile_pool(name="ps_keep", bufs=1, space="PSUM") as ps_keep,
            tc.tile_pool(name="ps_o", bufs=3, space="PSUM") as ps_o,
            tc.tile_pool(name="ps_y", bufs=3, space="PSUM") as ps_y,
        ):
            # HAM keepers in case the exchange outlasts pass B
            keep_ps = ps_keep.tile([128, C], f32)
            for i in range(N_KEEP_MID):
                nc.tensor.matmul(
                    keep_ps[:], ones_sb[:], zrow_sb[:],
                    start=(i == 0), stop=False, skip_group_check=True,
                )

            # ---- transpose probs -> block-diag pair lhsT (f32r) ----
            tr_ps = ps_tr.tile([64, NUM_HEADS, 64], f32)
            for h in range(NUM_HEADS):
                nc.tensor.transpose(tr_ps[:, h, :], probs2[:, h, :], ident[0:64, 0:64])
            for h in range(NUM_HEADS):
                p = h // 2
                off = (h % 2) * 64
                nc.vector.tensor_copy(
                    atnT[off:off + 64, p, off:off + 64], tr_ps[:, h, :]
                )

            # ---------------- phase 2: out + proj --------------------------
            def emit_out(c):
                outT_sb = sb.tile([128, N_CI, CHUNK], f32r, tag="outT", bufs=3, name=f"outT_{c}")
                for p in range(N_PAIRS):
                    o_ps = ps_o.tile([128, CHUNK], f32, tag="o", name=f"o_{c}_{p}")
                    nc.tensor.matmul(
                        o_ps[:], atnT[:, p, :], qT_all[:, p, c, :],
                        start=True, stop=True,
                    )
                    nc.scalar.copy(outT_sb[:, p, :], o_ps[:])
                return outT_sb

            outT_tiles = {0: emit_out(0), 1: emit_out(1)}
            for c in range(N_CHUNKS):
                if c + 2 < N_CHUNKS:
                    outT_tiles[c + 2] = emit_out(c + 2)
                outT_sb = outT_tiles.pop(c)
                for s in range(T_PER_CHUNK):
                    y_ps = ps_y.tile([128, C], f32, tag="y")
                    for k in range(N_CI):
                        nc.tensor.matmul(
                            y_ps[:],
                            outT_sb[:, k, s * TT:(s + 1) * TT],
                            wp_sb[:, k, :],
                            start=(k == 0), stop=(k == N_CI - 1),
                        )
                    y_sb = sb.tile([128, C], f32, tag="ysb", bufs=4)
                    nc.vector.tensor_add(y_sb[:], y_ps[:], bp_f32[:])
                    t0 = c * CHUNK + s * TT
                    nc.sync.dma_start(y[t0:t0 + TT, :], y_sb[:])

    nc.compile()
    return nc


def _get_nc():
    global _NC_CACHE
    if _NC_CACHE is None:
        _NC_CACHE = build_nc()
    return _NC_CACHE


def prep_inputs(x, Wqkv, Wproj, bproj):
    x = np.ascontiguousarray(np.asarray(x, dtype=np.float32))
    Wqkv = np.asarray(Wqkv, dtype=np.float32)
    Wproj = np.asarray(Wproj, dtype=np.float32)
    bproj = np.asarray(bproj, dtype=np.float32)

    xf = x.reshape(B, D * H * W, C)
    wq = np.ascontiguousarray(Wqkv[0:C].T)
    wk = Wqkv[C:2 * C] * np.float32(SCALE)
    wv = Wqkv[2 * C:3 * C]
    wkv = np.ascontiguousarray(np.concatenate([wk, wv], axis=0).T)
    wp = np.ascontiguousarray(Wproj.T)
    bp = np.ascontiguousarray(bproj.reshape(1, C))

    in_maps = []
    for i in range(N_CORES):
        b = i // 2
        t0 = (i % 2) * N_LOC
        own = xf[b, t0:t0 + N_LOC, :]
        pair = xf[b, N_LOC - t0:2 * N_LOC - t0, :]
        xTl = np.ascontiguousarray(np.concatenate([own, pair], axis=0).T)
        in_maps.append({"xT": xTl, "wq": wq, "wkv": wkv, "wp": wp, "bp": bp})
    return in_maps


def gather_output(results):
    parts = [np.asarray(results[i]["y"]) for i in range(N_CORES)]
    return np.concatenate(parts, axis=0).reshape(B, D, H, W, C)


def kernel(x, Wqkv, Wproj, bproj, _trace=False, _tmpdir=None):
    nc = _get_nc()
    in_maps = prep_inputs(x, Wqkv, Wproj, bproj)
    res = run_bass_kernel_spmd(
        nc, in_maps, list(range(N_CORES)), trace=_trace, tmpdir=_tmpdir
    )
    out = gather_output(res.results)
    if _trace:
        kernel.last_exec_time_ns = res.exec_time_ns
        kernel.last_results = res
    return out



# revision 4
# speedup vs baseline: 1.4952x; 1.4952x over previous
"""ChannelAttention (B,D,H,W,C = 4,8,32,32,512; 8 heads, head_dim 64) on 8
Trainium2 NeuronCores, Bass/Tile SPMD. Fully data-parallel, zero cross-core
communication; cores (2j, 2j+1) split batch j's 8192 tokens in half.

Algebraic restructuring vs the straightforward schedule: channel attention's
per-head Gram matrix is a bilinear form of the input Gram,
    G_h = (Wk_h x^T) (x Wv_h^T) = Wk_h (x^T x) Wv_h^T,
so one S = x^T x (contract 8192 tokens, 512x512 out) replaces the k|v
projection of the whole batch; and because attn is only 64x64 per head, the
entire q -> out -> proj chain folds into a single fused 512x512 matrix
    M = sum_h Wq_h^T attn_h^T WpT_h,  y = x @ M + b,
so the per-token epilogue is one matmul instead of three.

Schedule per core:
  S     : stream token-major x (bf16), accumulate upper-triangle blocks of
          S = x^T x into PSUM (4 banks, N = 512/384/256/128 moving), HAM
          keeper matmuls warm the PE during the DMA fill. Mirror the 6
          lower blocks via PE transposes.
  chain : AT = S @ WkT (scale folded), G per head-pair via 256-wide f32r
          windows, rowwise softmax (DVE/ACT), P = attn^T @ WpT (block-diag
          pair packing, no transpose needed), M = Wq^T @ P.
  fused : stream own-half x^T (f32r), y = x @ M + bias, DMA out.

Numerics: S in bf16 inputs (coherent x quantization, fp32 PSUM accumulate),
everything else f32r with fp32 accumulation. End-to-end L2 relative error vs
the fp32 reference: ~2e-3.
"""

import numpy as np
import ml_dtypes
from contextlib import ExitStack

import concourse.bass as bass
import concourse.mybir as mybir
import concourse.tile as tile
from concourse import bacc
from concourse.bass_utils import run_bass_kernel_spmd
from concourse.masks import make_identity

B, D, H, W, C = 4, 8, 32, 32, 512
NUM_HEADS = 8
HEAD_DIM = C // NUM_HEADS
SCALE = HEAD_DIM ** -0.5
N_TOK = B * D * H * W
N_CORES = 8
N_LOC = N_TOK // N_CORES        # 4096 output tokens per core
N_BTOK = 2 * N_LOC              # 8192 tokens per batch (S contraction)
N_STILE = N_BTOK // 128         # 64 token tiles for S
TPC = 8                         # token tiles per x DMA chunk
N_XCH = N_STILE // TPC
CHUNK = 512                     # fused-pass chunk (tokens)
N_CHUNKS = N_LOC // CHUNK
N_CI = C // 128                 # 4 channel blocks
N_PAIRS = NUM_HEADS // 2

f32 = mybir.dt.float32
f32r = mybir.dt.float32r
bf16 = mybir.dt.bfloat16

N_KEEP_START = 16
N_KEEP_MID = 12

_NC_CACHE = None


def build_nc():
    nc = bacc.Bacc(num_devices=N_CORES)

    xk = nc.declare_dram_parameter("xk", [N_BTOK, C], bf16, isOutput=False)
    xT = nc.declare_dram_parameter("xT", [C, N_LOC], f32r, isOutput=False)
    wkT = nc.declare_dram_parameter("wkT", [C, C], f32r, isOutput=False)
    wvT = nc.declare_dram_parameter("wvT", [C, C], f32r, isOutput=False)
    wqh = nc.declare_dram_parameter("wqh", [C, C], f32r, isOutput=False)
    wpT = nc.declare_dram_parameter("wpT", [C, C], f32r, isOutput=False)
    bp = nc.declare_dram_parameter("bp", [1, C], f32r, isOutput=False)
    y = nc.declare_dram_parameter("y", [N_LOC, C], f32, isOutput=True)

    xk_v = xk.rearrange("(a p) c -> p a c", p=128)
    xT_v = xT.rearrange("(a p) n -> p a n", p=128)
    wkT_v = wkT.rearrange("(a p) f -> p a f", p=128)
    wvT_v = wvT.rearrange("(a p) f -> p a f", p=128)
    wqh_v = wqh.rearrange("(a p) f -> p a f", p=128)
    wpT_v = wpT.rearrange("(a p) f -> p a f", p=128)

    with tile.TileContext(nc) as tc, ExitStack() as ctx:
        const = ctx.enter_context(tc.tile_pool(name="const", bufs=1))
        persist = ctx.enter_context(tc.tile_pool(name="persist", bufs=1))
        sb = ctx.enter_context(tc.tile_pool(name="sb", bufs=2))

        wkT_sb = const.tile([128, N_CI, C], f32r)
        nc.sync.dma_start(wkT_sb[:], wkT_v[:])
        wvT_sb = const.tile([128, N_CI, C], f32r)
        nc.sync.dma_start(wvT_sb[:], wvT_v[:])

        ones_f32 = const.tile([1, 128], f32)
        nc.vector.memset(ones_f32[:], 1.0)
        ones_sb = const.tile([1, 128], f32r)
        nc.vector.tensor_copy(ones_sb[:], ones_f32[:])
        zrow_f32 = const.tile([1, 512], f32)
        nc.vector.memset(zrow_f32[:], 0.0)
        zrow_sb = const.tile([1, 512], f32r)
        nc.vector.tensor_copy(zrow_sb[:], zrow_f32[:])
        ident = const.tile([128, 128], f32)
        make_identity(nc, ident[:])
        ident_r = const.tile([128, 128], f32r)
        nc.vector.tensor_copy(ident_r[:], ident[:])

        S_sb = persist.tile([128, N_CI, C], f32r)
        red_sb = persist.tile([128, N_PAIRS, 64], f32)

        # ---------------- S = x^T x (upper triangle) -----------------------
        s_w = [C - 128 * i for i in range(N_CI)]  # 512, 384, 256, 128
        with (
            tc.tile_pool(name="ps_s", bufs=1, space="PSUM") as ps_s,
            tc.tile_pool(name="ps_tr", bufs=1, space="PSUM") as ps_tr,
        ):
            s_ps = [ps_s.tile([128, s_w[i]], f32, name=f"s{i}") for i in range(N_CI)]
            # bank-wide has_written seed + HAM warm-up during the DMA fill
            for i in range(N_KEEP_START):
                cb = i % N_CI
                nc.tensor.matmul(
                    s_ps[cb][:], ones_sb[:], zrow_sb[:, 0:s_w[cb]],
                    start=(i < N_CI), stop=False, skip_group_check=True,
                )

            for ch in range(N_XCH):
                xt = sb.tile([128, TPC, C], bf16, tag="xtok")
                nc.sync.dma_start(xt[:], xk_v[:, ch * TPC:(ch + 1) * TPC, :])
                for t in range(TPC):
                    last = (ch == N_XCH - 1 and t == TPC - 1)
                    for cb in range(N_CI):
                        nc.tensor.matmul(
                            s_ps[cb][:],
                            xt[:, t, cb * 128:(cb + 1) * 128],
                            xt[:, t, cb * 128:C],
                            start=False, stop=last, skip_group_check=True,
                        )

            # upper blocks -> S_sb (f32r), split across vector/scalar engines
            for cb in range(N_CI):
                if cb % 2 == 0:
                    nc.vector.tensor_copy(S_sb[:, cb, cb * 128:C], s_ps[cb][:])
                else:
                    nc.scalar.copy(S_sb[:, cb, cb * 128:C], s_ps[cb][:])

            # mirror the 6 lower blocks via PE transpose of upper (j, i)
            tr_ps = ps_tr.tile([128, 6, 128], f32r)
            idx = 0
            tr_slots = []
            for i in range(N_CI):
                for j in range(i):
                    nc.tensor.transpose(
                        tr_ps[:, idx, :], S_sb[:, j, i * 128:(i + 1) * 128],
                        ident_r[:],
                    )
                    tr_slots.append((i, j, idx))
                    idx += 1
            for (i, j, k) in tr_slots:
                nc.vector.tensor_copy(S_sb[:, i, j * 128:(j + 1) * 128], tr_ps[:, k, :])

        # ---------------- AT = S @ WkT ; G = AT-blocks @ WvT-blocks --------
        with (
            tc.tile_pool(name="ps_at", bufs=1, space="PSUM") as ps_at,
            tc.tile_pool(name="ps_g", bufs=1, space="PSUM") as ps_g,
        ):
            AT_ps = ps_at.tile([128, N_CI, C], f32)
            for j in range(N_CI):
                for i in range(N_CI):
                    nc.tensor.matmul(
                        AT_ps[:, j, :],
                        S_sb[:, i, j * 128:(j + 1) * 128],
                        wkT_sb[:, i, :],
                        start=(i == 0), stop=(i == N_CI - 1),
                    )
            AT_sb = persist.tile([128, N_CI, C], f32r)
            for j in range(N_CI):
                if j % 2 == 0:
                    nc.vector.tensor_copy(AT_sb[:, j, :], AT_ps[:, j, :])
                else:
                    nc.scalar.copy(AT_sb[:, j, :], AT_ps[:, j, :])

            # G per pair in a 256-wide window (f32r full rate needs N>=256)
            g_w = [min(p * 128, C - 256) for p in range(N_PAIRS)]
            G_ps = ps_g.tile([128, N_PAIRS, 256], f32)
            for p in range(N_PAIRS):
                for j in range(N_CI):
                    nc.tensor.matmul(
                        G_ps[:, p, :],
                        AT_sb[:, j, p * 128:(p + 1) * 128],
                        wvT_sb[:, j, g_w[p]:g_w[p] + 256],
                        start=(j == 0), stop=(j == N_CI - 1),
                    )

            # pack 8 useful 64x64 blocks -> red_sb[d + 64*(h%2), h//2, :]
            for h in range(NUM_HEADS):
                p = h // 2
                row0 = (h % 2) * 64
                col0 = p * 128 + row0 - g_w[p]
                nc.vector.tensor_copy(
                    red_sb[row0:row0 + 64, p, :],
                    G_ps[row0:row0 + 64, p, col0:col0 + 64],
                )

        # weights for the M build / fused pass (after pass-A DMAs drain)
        wqh_sb = const.tile([128, N_CI, C], f32r)
        nc.sync.dma_start(wqh_sb[:], wqh_v[:])
        wpT_sb = const.tile([128, N_CI, C], f32r)
        nc.sync.dma_start(wpT_sb[:], wpT_v[:])
        bp_f32 = const.tile([128, C], f32)
        bp_bcast = bass.AP(
            tensor=bp[:].bitcast(f32).tensor,
            offset=0,
            ap=[[0, 128], [1, C]],
        )
        nc.sync.dma_start(bp_f32[:], bp_bcast)

        with (
            tc.tile_pool(name="ps_p", bufs=1, space="PSUM") as ps_p,
            tc.tile_pool(name="ps_m", bufs=1, space="PSUM") as ps_m,
        ):
            P_ps = ps_p.tile([128, N_PAIRS, C], f32)
            # HAM keepers bridge the PE gap while softmax runs on DVE/ACT
            for i in range(N_KEEP_MID):
                nc.tensor.matmul(
                    P_ps[:, i % N_PAIRS, :], ones_sb[:], zrow_sb[:],
                    start=(i < N_PAIRS), stop=False, skip_group_check=True,
                )

            # ---- softmax over e on [128, pair, 64] ----
            nmax = sb.tile([128, N_PAIRS, 1], f32, tag="nmax")
            nc.vector.reduce_max(nmax[:], red_sb[:], axis=mybir.AxisListType.X, negate=True)
            shifted = sb.tile([128, N_PAIRS, 64], f32, tag="shifted")
            nc.vector.tensor_add(shifted[:], red_sb[:], nmax.broadcast_to([128, N_PAIRS, 64]))
            expd = sb.tile([128, N_PAIRS, 64], f32, tag="expd")
            nc.scalar.activation(expd[:], shifted[:], mybir.ActivationFunctionType.Exp)
            ssum = sb.tile([128, N_PAIRS, 1], f32, tag="ssum")
            nc.vector.reduce_sum(ssum[:], expd[:], axis=mybir.AxisListType.X)
            rsum = sb.tile([128, N_PAIRS, 1], f32, tag="rsum")
            nc.vector.reciprocal(rsum[:], ssum[:])
            probs = sb.tile([128, N_PAIRS, 64], f32, tag="probs")
            nc.vector.tensor_mul(probs[:], expd[:], rsum.broadcast_to([128, N_PAIRS, 64]))

            # block-diag pair lhsT with attn[d, e], d on partitions (no
            # transpose: P-stage contracts d)
            zro = sb.tile([128, N_PAIRS, 128], f32, tag="zro")
            nc.vector.memset(zro[:], 0.0)
            atn_sb = persist.tile([128, N_PAIRS, 128], f32r)
            nc.vector.tensor_copy(atn_sb[:], zro[:])
            for p in range(N_PAIRS):
                nc.vector.tensor_copy(atn_sb[0:64, p, 0:64], probs[0:64, p, :])
                nc.vector.tensor_copy(atn_sb[64:128, p, 64:128], probs[64:128, p, :])

            # ---- P = attn^T @ WpT (pair block-diag) ; M = Wq^T @ P --------
            for p in range(N_PAIRS):
                nc.tensor.matmul(
                    P_ps[:, p, :], atn_sb[:, p, :], wpT_sb[:, p, :],
                    start=True, stop=True, skip_group_check=True,
                )
            P_sb = persist.tile([128, N_PAIRS, C], f32r)
            for p in range(N_PAIRS):
                if p % 2 == 0:
                    nc.vector.tensor_copy(P_sb[:, p, :], P_ps[:, p, :])
                else:
                    nc.scalar.copy(P_sb[:, p, :], P_ps[:, p, :])

            M_ps = ps_m.tile([128, N_CI, C], f32)
            for cb in range(N_CI):
                for j in range(N_CI):
                    nc.tensor.matmul(
                        M_ps[:, cb, :],
                        wqh_sb[:, j, cb * 128:(cb + 1) * 128],
                        P_sb[:, j, :],
                        start=(j == 0), stop=(j == N_CI - 1),
                    )
            M_sb = persist.tile([128, N_CI, C], f32r)
            for cb in range(N_CI):
                if cb % 2 == 0:
                    nc.vector.tensor_copy(M_sb[:, cb, :], M_ps[:, cb, :])
                else:
                    nc.scalar.copy(M_sb[:, cb, :], M_ps[:, cb, :])

        # ---------------- fused pass: y = x @ M + b ------------------------
        with tc.tile_pool(name="ps_y", bufs=3, space="PSUM") as ps_y:
            for c in range(N_CHUNKS):
                xt = sb.tile([128, N_CI, CHUNK], f32r, tag="xT")
                nc.sync.dma_start(xt[:], xT_v[:, :, c * CHUNK:(c + 1) * CHUNK])
                for s in range(CHUNK // 128):
                    y_ps = ps_y.tile([128, C], f32, tag="y")
                    for cb in range(N_CI):
                        nc.tensor.matmul(
                            y_ps[:],
                            xt[:, cb, s * 128:(s + 1) * 128],
                            M_sb[:, cb, :],
                            start=(cb == 0), stop=(cb == N_CI - 1),
                        )
                    y_sb = sb.tile([128, C], f32, tag="ysb", bufs=4)
                    nc.vector.tensor_add(y_sb[:], y_ps[:], bp_f32[:])
                    t0 = c * CHUNK + s * 128
                    nc.sync.dma_start(y[t0:t0 + 128, :], y_sb[:])

    nc.compile()
    return nc


def _get_nc():
    global _NC_CACHE
    if _NC_CACHE is None:
        _NC_CACHE = build_nc()
    return _NC_CACHE


def prep_inputs(x, Wqkv, Wproj, bproj):
    x = np.ascontiguousarray(np.asarray(x, dtype=np.float32))
    Wqkv = np.asarray(Wqkv, dtype=np.float32)
    Wproj = np.asarray(Wproj, dtype=np.float32)
    bproj = np.asarray(bproj, dtype=np.float32)

    xf = x.reshape(B, N_BTOK, C)
    wq = Wqkv[0:C]                               # [he, c]
    wk_s = Wqkv[C:2 * C] * np.float32(SCALE)     # [kd, c], scale folded
    wv = Wqkv[2 * C:3 * C]                       # [vd, c]
    wkT = np.ascontiguousarray(wk_s.T)           # [c, kd]
    wvT = np.ascontiguousarray(wv.T)             # [c, vd]
    wpT = np.ascontiguousarray(Wproj.T)          # [hd, c'']
    bp = np.ascontiguousarray(bproj.reshape(1, C))

    xk_b = [np.ascontiguousarray(xf[b]).astype(ml_dtypes.bfloat16) for b in range(B)]

    in_maps = []
    for i in range(N_CORES):
        b = i // 2
        t0 = (i % 2) * N_LOC
        xTl = np.ascontiguousarray(xf[b, t0:t0 + N_LOC, :].T)
        in_maps.append({
            "xk": xk_b[b], "xT": xTl, "wkT": wkT, "wvT": wvT,
            "wqh": wq, "wpT": wpT, "bp": bp,
        })
    return in_maps


def gather_output(results):
    parts = [np.asarray(results[i]["y"]) for i in range(N_CORES)]
    return np.concatenate(parts, axis=0).reshape(B, D, H, W, C)


def kernel(x, Wqkv, Wproj, bproj, _trace=False, _tmpdir=None):
    nc = _get_nc()
    in_maps = prep_inputs(x, Wqkv, Wproj, bproj)
    res = run_bass_kernel_spmd(
        nc, in_maps, list(range(N_CORES)), trace=_trace, tmpdir=_tmpdir
    )
    out = gather_output(res.results)
    if _trace:
        kernel.last_exec_time_ns = res.exec_time_ns
        kernel.last_results = res
    return out


# revision 5
# speedup vs baseline: 1.8537x; 1.2398x over previous
"""ChannelAttention (B,D,H,W,C = 4,8,32,32,512; 8 heads, head_dim 64) on 8
Trainium2 NeuronCores, Bass/Tile SPMD. Fully data-parallel, zero cross-core
communication; cores (2j, 2j+1) split batch j's 8192 tokens in half.

Algebraic restructuring vs the straightforward schedule: channel attention's
per-head Gram matrix is a bilinear form of the input Gram,
    G_h = (Wk_h x^T) (x Wv_h^T) = Wk_h (x^T x) Wv_h^T,
so one S = x^T x (contract 8192 tokens, 512x512 out) replaces the k|v
projection of the whole batch; and because attn is only 64x64 per head, the
entire q -> out -> proj chain folds into a single fused 512x512 matrix
    M = sum_h Wq_h^T attn_h^T WpT_h,  y = x @ M + b,
so the per-token epilogue is one matmul instead of three.

All DRAM streams are host pre-permuted to partition-major [128, ...]
layouts so every DMA moves 4-8 KiB contiguous per partition line.

Schedule per core:
  S     : stream token-major x (bf16), accumulate upper-triangle blocks of
          S = x^T x into PSUM (N = 512/384/256/128 moving), HAM keeper
          matmuls warm the PE during the DMA fill. Mirror the 6 lower
          blocks via PE transposes.
  chain : AT = S @ WkT (scale folded), G per head-pair via 256-wide f32r
          windows, rowwise softmax (DVE/ACT; exp table pre-warmed), P =
          attn^T @ WpT (block-diag pair packing, no transpose needed),
          M = Wq^T @ P.
  fused : stream own-half x^T (bf16), y = x @ M + bias, batched DMA out.

Numerics: S and the fused pass take bf16 inputs (fp32 PSUM accumulation),
the small chain matmuls are f32r. End-to-end L2 relative error vs the fp32
reference: ~4e-3.
"""

import numpy as np
import ml_dtypes
from contextlib import ExitStack

import concourse.bass as bass
import concourse.mybir as mybir
import concourse.tile as tile
from concourse import bacc
from concourse.bass_utils import run_bass_kernel_spmd
from concourse.masks import make_identity

B, D, H, W, C = 4, 8, 32, 32, 512
NUM_HEADS = 8
HEAD_DIM = C // NUM_HEADS
SCALE = HEAD_DIM ** -0.5
N_TOK = B * D * H * W
N_CORES = 8
N_LOC = N_TOK // N_CORES        # 4096 output tokens per core
N_BTOK = 2 * N_LOC              # 8192 tokens per batch (S contraction)
N_STILE = N_BTOK // 128         # 64 token tiles for S
CHUNK = 512                     # fused-pass chunk (tokens)
N_CHUNKS = N_LOC // CHUNK
N_CI = C // 128                 # 4 channel blocks
N_PAIRS = NUM_HEADS // 2

# x DMA chunking: two small lead chunks so the PE starts sooner
S_CHUNKS = [(0, 4), (4, 4)] + [(a, 8) for a in range(8, N_STILE, 8)]

f32 = mybir.dt.float32
f32r = mybir.dt.float32r
bf16 = mybir.dt.bfloat16

N_KEEP_START = 18
N_KEEP_MID = 12

_NC_CACHE = None


def build_nc():
    nc = bacc.Bacc(num_devices=N_CORES)

    xk = nc.declare_dram_parameter("xk", [128, N_STILE, C], bf16, isOutput=False)
    xT = nc.declare_dram_parameter("xT", [128, N_CHUNKS, N_CI, CHUNK], bf16, isOutput=False)
    wkT = nc.declare_dram_parameter("wkT", [128, N_CI, C], f32r, isOutput=False)
    wvT = nc.declare_dram_parameter("wvT", [128, N_CI, C], f32r, isOutput=False)
    wqh = nc.declare_dram_parameter("wqh", [128, N_CI, C], f32r, isOutput=False)
    wpT = nc.declare_dram_parameter("wpT", [128, N_CI, C], f32r, isOutput=False)
    bp = nc.declare_dram_parameter("bp", [1, C], f32r, isOutput=False)
    y = nc.declare_dram_parameter("y", [128, N_CHUNKS, N_CI, C], f32, isOutput=True)

    with tile.TileContext(nc) as tc, ExitStack() as ctx:
        const = ctx.enter_context(tc.tile_pool(name="const", bufs=1))
        persist = ctx.enter_context(tc.tile_pool(name="persist", bufs=1))
        sb = ctx.enter_context(tc.tile_pool(name="sb", bufs=2))

        wkT_sb = const.tile([128, N_CI, C], f32r)
        nc.sync.dma_start(wkT_sb[:], wkT[:])
        wvT_sb = const.tile([128, N_CI, C], f32r)
        nc.sync.dma_start(wvT_sb[:], wvT[:])

        ones_f32 = const.tile([1, 128], f32)
        nc.vector.memset(ones_f32[:], 1.0)
        ones_sb = const.tile([1, 128], f32r)
        nc.vector.tensor_copy(ones_sb[:], ones_f32[:])
        zrow_f32 = const.tile([1, 512], f32)
        nc.vector.memset(zrow_f32[:], 0.0)
        zrow_sb = const.tile([1, 512], f32r)
        nc.vector.tensor_copy(zrow_sb[:], zrow_f32[:])
        ident = const.tile([128, 128], f32)
        make_identity(nc, ident[:])
        ident_r = const.tile([128, 128], f32r)
        nc.vector.tensor_copy(ident_r[:], ident[:])
        # pre-warm the scalar engine's Exp table off the critical chain
        warm = const.tile([1, 2], f32)
        nc.scalar.activation(warm[:], ones_f32[:, 0:2], mybir.ActivationFunctionType.Exp)

        S_sb = persist.tile([128, N_CI, C], f32r)
        red_sb = persist.tile([128, N_PAIRS, 64], f32)

        # ---------------- S = x^T x (upper triangle) -----------------------
        s_w = [C - 128 * i for i in range(N_CI)]  # 512, 384, 256, 128
        with (
            tc.tile_pool(name="ps_s", bufs=1, space="PSUM") as ps_s,
            tc.tile_pool(name="ps_tr", bufs=1, space="PSUM") as ps_tr,
        ):
            s_ps = [ps_s.tile([128, s_w[i]], f32, name=f"s{i}") for i in range(N_CI)]
            # bank-wide has_written seed + HAM warm-up during the DMA fill
            for i in range(N_KEEP_START):
                cb = i % N_CI
                nc.tensor.matmul(
                    s_ps[cb][:], ones_sb[:], zrow_sb[:, 0:s_w[cb]],
                    start=(i < N_CI), stop=False, skip_group_check=True,
                )

            for (a0, ntl) in S_CHUNKS:
                xt = sb.tile([128, ntl, C], bf16, tag="xtok", name=f"xt{a0}")
                nc.sync.dma_start(xt[:], xk[:, a0:a0 + ntl, :])
                for t in range(ntl):
                    last = (a0 + t == N_STILE - 1)
                    for cb in range(N_CI):
                        nc.tensor.matmul(
                            s_ps[cb][:],
                            xt[:, t, cb * 128:(cb + 1) * 128],
                            xt[:, t, cb * 128:C],
                            start=False, stop=last, skip_group_check=True,
                        )

            # upper blocks -> S_sb (f32r), split across vector/scalar engines
            for cb in range(N_CI):
                if cb % 2 == 0:
                    nc.vector.tensor_copy(S_sb[:, cb, cb * 128:C], s_ps[cb][:])
                else:
                    nc.scalar.copy(S_sb[:, cb, cb * 128:C], s_ps[cb][:])

            # mirror the 6 lower blocks via PE transpose of upper (j, i)
            tr_ps = ps_tr.tile([128, 6, 128], f32r)
            idx = 0
            tr_slots = []
            for i in range(N_CI):
                for j in range(i):
                    nc.tensor.transpose(
                        tr_ps[:, idx, :], S_sb[:, j, i * 128:(i + 1) * 128],
                        ident_r[:],
                    )
                    tr_slots.append((i, j, idx))
                    idx += 1
            for (i, j, k) in tr_slots:
                nc.vector.tensor_copy(S_sb[:, i, j * 128:(j + 1) * 128], tr_ps[:, k, :])

        # ---------------- AT = S @ WkT ; G = AT-blocks @ WvT-blocks --------
        with (
            tc.tile_pool(name="ps_at", bufs=1, space="PSUM") as ps_at,
            tc.tile_pool(name="ps_g", bufs=1, space="PSUM") as ps_g,
            tc.tile_pool(name="ps_k1", bufs=1, space="PSUM") as ps_k1,
        ):
            # keepers bridge the S_sb copy gap
            k1_ps = ps_k1.tile([128, C], f32)
            for i in range(6):
                nc.tensor.matmul(
                    k1_ps[:], ones_sb[:], zrow_sb[:],
                    start=(i == 0), stop=False, skip_group_check=True,
                )

            AT_ps = ps_at.tile([128, N_CI, C], f32)
            for j in range(N_CI):
                for i in range(N_CI):
                    nc.tensor.matmul(
                        AT_ps[:, j, :],
                        S_sb[:, i, j * 128:(j + 1) * 128],
                        wkT_sb[:, i, :],
                        start=(i == 0), stop=(i == N_CI - 1),
                    )
            AT_sb = persist.tile([128, N_CI, C], f32r)
            for j in range(N_CI):
                if j % 2 == 0:
                    nc.vector.tensor_copy(AT_sb[:, j, :], AT_ps[:, j, :])
                else:
                    nc.scalar.copy(AT_sb[:, j, :], AT_ps[:, j, :])

            # keepers bridge the AT_sb copy gap
            for i in range(4):
                nc.tensor.matmul(
                    k1_ps[:], ones_sb[:], zrow_sb[:],
                    start=False, stop=(i == 3), skip_group_check=True,
                )

            # G per pair in a 256-wide window (f32r full rate needs N>=256)
            g_w = [min(p * 128, C - 256) for p in range(N_PAIRS)]
            G_ps = ps_g.tile([128, N_PAIRS, 256], f32)
            for p in range(N_PAIRS):
                for j in range(N_CI):
                    nc.tensor.matmul(
                        G_ps[:, p, :],
                        AT_sb[:, j, p * 128:(p + 1) * 128],
                        wvT_sb[:, j, g_w[p]:g_w[p] + 256],
                        start=(j == 0), stop=(j == N_CI - 1),
                    )

            # pack 8 useful 64x64 blocks -> red_sb[d + 64*(h%2), h//2, :]
            for h in range(NUM_HEADS):
                p = h // 2
                row0 = (h % 2) * 64
                col0 = p * 128 + row0 - g_w[p]
                nc.vector.tensor_copy(
                    red_sb[row0:row0 + 64, p, :],
                    G_ps[row0:row0 + 64, p, col0:col0 + 64],
                )

        # weights for the M build / fused pass (after pass-A DMAs drain)
        wqh_sb = const.tile([128, N_CI, C], f32r)
        nc.sync.dma_start(wqh_sb[:], wqh[:])
        wpT_sb = const.tile([128, N_CI, C], f32r)
        nc.sync.dma_start(wpT_sb[:], wpT[:])
        bp_f32 = const.tile([128, C], f32)
        bp_bcast = bass.AP(
            tensor=bp[:].bitcast(f32).tensor,
            offset=0,
            ap=[[0, 128], [1, C]],
        )
        nc.sync.dma_start(bp_f32[:], bp_bcast)

        with (
            tc.tile_pool(name="ps_p", bufs=1, space="PSUM") as ps_p,
            tc.tile_pool(name="ps_m", bufs=1, space="PSUM") as ps_m,
        ):
            P_ps = ps_p.tile([128, N_PAIRS, C], f32)
            # HAM keepers bridge the PE gap while softmax runs on DVE/ACT
            for i in range(N_KEEP_MID):
                nc.tensor.matmul(
                    P_ps[:, i % N_PAIRS, :], ones_sb[:], zrow_sb[:],
                    start=(i < N_PAIRS), stop=False, skip_group_check=True,
                )

            # ---- softmax over e on [128, pair, 64] ----
            nmax = sb.tile([128, N_PAIRS, 1], f32, tag="nmax")
            nc.vector.reduce_max(nmax[:], red_sb[:], axis=mybir.AxisListType.X, negate=True)
            shifted = sb.tile([128, N_PAIRS, 64], f32, tag="shifted")
            nc.vector.tensor_add(shifted[:], red_sb[:], nmax.broadcast_to([128, N_PAIRS, 64]))
            expd = sb.tile([128, N_PAIRS, 64], f32, tag="expd")
            nc.scalar.activation(expd[:], shifted[:], mybir.ActivationFunctionType.Exp)
            ssum = sb.tile([128, N_PAIRS, 1], f32, tag="ssum")
            nc.vector.reduce_sum(ssum[:], expd[:], axis=mybir.AxisListType.X)
            rsum = sb.tile([128, N_PAIRS, 1], f32, tag="rsum")
            nc.vector.reciprocal(rsum[:], ssum[:])
            probs = sb.tile([128, N_PAIRS, 64], f32, tag="probs")
            nc.vector.tensor_mul(probs[:], expd[:], rsum.broadcast_to([128, N_PAIRS, 64]))

            # block-diag pair lhsT with attn[d, e], d on partitions (no
            # transpose: P-stage contracts d)
            zro = sb.tile([128, N_PAIRS, 128], f32, tag="zro")
            nc.vector.memset(zro[:], 0.0)
            atn_sb = persist.tile([128, N_PAIRS, 128], f32r)
            nc.vector.tensor_copy(atn_sb[:], zro[:])
            for p in range(N_PAIRS):
                nc.vector.tensor_copy(atn_sb[0:64, p, 0:64], probs[0:64, p, :])
                nc.vector.tensor_copy(atn_sb[64:128, p, 64:128], probs[64:128, p, :])

            # ---- P = attn^T @ WpT (pair block-diag) ; M = Wq^T @ P --------
            for p in range(N_PAIRS):
                nc.tensor.matmul(
                    P_ps[:, p, :], atn_sb[:, p, :], wpT_sb[:, p, :],
                    start=True, stop=True, skip_group_check=True,
                )
            P_sb = persist.tile([128, N_PAIRS, C], f32r)
            for p in range(N_PAIRS):
                if p % 2 == 0:
                    nc.vector.tensor_copy(P_sb[:, p, :], P_ps[:, p, :])
                else:
                    nc.scalar.copy(P_sb[:, p, :], P_ps[:, p, :])

            M_ps = ps_m.tile([128, N_CI, C], f32)
            for cb in range(N_CI):
                for j in range(N_CI):
                    nc.tensor.matmul(
                        M_ps[:, cb, :],
                        wqh_sb[:, j, cb * 128:(cb + 1) * 128],
                        P_sb[:, j, :],
                        start=(j == 0), stop=(j == N_CI - 1),
                    )
            M_sb = persist.tile([128, N_CI, C], bf16)
            for cb in range(N_CI):
                if cb % 2 == 0:
                    nc.vector.tensor_copy(M_sb[:, cb, :], M_ps[:, cb, :])
                else:
                    nc.scalar.copy(M_sb[:, cb, :], M_ps[:, cb, :])

        # ---------------- fused pass: y = x @ M + b ------------------------
        with tc.tile_pool(name="ps_y", bufs=3, space="PSUM") as ps_y:
            for c in range(N_CHUNKS):
                xt = sb.tile([128, N_CI, CHUNK], bf16, tag="xT", bufs=3)
                nc.sync.dma_start(xt[:], xT[:, c, :, :])
                y_acc = sb.tile([128, N_CI, C], f32, tag="yacc")
                for s in range(CHUNK // 128):
                    y_ps = ps_y.tile([128, C], f32, tag="y")
                    for cb in range(N_CI):
                        nc.tensor.matmul(
                            y_ps[:],
                            xt[:, cb, s * 128:(s + 1) * 128],
                            M_sb[:, cb, :],
                            start=(cb == 0), stop=(cb == N_CI - 1),
                        )
                    nc.vector.tensor_add(y_acc[:, s, :], y_ps[:], bp_f32[:])
                nc.sync.dma_start(y[:, c, :, :], y_acc[:])

    nc.compile()
    return nc


def _get_nc():
    global _NC_CACHE
    if _NC_CACHE is None:
        _NC_CACHE = build_nc()
    return _NC_CACHE


def prep_inputs(x, Wqkv, Wproj, bproj):
    x = np.ascontiguousarray(np.asarray(x, dtype=np.float32))
    Wqkv = np.asarray(Wqkv, dtype=np.float32)
    Wproj = np.asarray(Wproj, dtype=np.float32)
    bproj = np.asarray(bproj, dtype=np.float32)

    xf = x.reshape(B, N_BTOK, C)
    wq = Wqkv[0:C]                               # [he, c]
    wk_s = Wqkv[C:2 * C] * np.float32(SCALE)     # [kd, c], scale folded
    wv = Wqkv[2 * C:3 * C]                       # [vd, c]

    def pmaj(a):  # [512, 512] -> [128, 4, 512] partition-major
        return np.ascontiguousarray(a.reshape(N_CI, 128, C).transpose(1, 0, 2))

    wkT = pmaj(wk_s.T)
    wvT = pmaj(wv.T)
    wqh = pmaj(wq)
    wpT = pmaj(Wproj.T)
    bp = np.ascontiguousarray(bproj.reshape(1, C))

    # token-major x, pre-permuted: [128, 64 tiles, 512], token = a*128 + p
    xk_b = [
        np.ascontiguousarray(
            xf[b].reshape(N_STILE, 128, C).transpose(1, 0, 2)
        ).astype(ml_dtypes.bfloat16)
        for b in range(B)
    ]

    in_maps = []
    for i in range(N_CORES):
        b = i // 2
        t0 = (i % 2) * N_LOC
        # x^T pre-permuted: [128, chunk, cb, tok], c = cb*128 + p
        xTl = np.ascontiguousarray(
            xf[b, t0:t0 + N_LOC, :].T
            .reshape(N_CI, 128, N_CHUNKS, CHUNK).transpose(1, 2, 0, 3)
        ).astype(ml_dtypes.bfloat16)
        in_maps.append({
            "xk": xk_b[b], "xT": xTl, "wkT": wkT, "wvT": wvT,
            "wqh": wqh, "wpT": wpT, "bp": bp,
        })
    return in_maps


def gather_output(results):
    parts = []
    for i in range(N_CORES):
        yl = np.asarray(results[i]["y"])  # [128, chunk, s, 512]
        parts.append(yl.transpose(1, 2, 0, 3).reshape(N_LOC, C))
    return np.concatenate(parts, axis=0).reshape(B, D, H, W, C)


def kernel(x, Wqkv, Wproj, bproj, _trace=False, _tmpdir=None):
    nc = _get_nc()
    in_maps = prep_inputs(x, Wqkv, Wproj, bproj)
    res = run_bass_kernel_spmd(
        nc, in_maps, list(range(N_CORES)), trace=_trace, tmpdir=_tmpdir
    )
    out = gather_output(res.results)
    if _trace:
        kernel.last_exec_time_ns = res.exec_time_ns
        kernel.last_results = res
    return out
